# revision 1
# baseline (speedup 1.0000x reference)
"""Trainium2 Bass kernel for nn_Block_65755949302136 (dense transformer block).

Sharding: 8 cores = 2 (batch) x 4 (tensor-parallel ranks). Each rank owns 4
heads (2 sloped-ALiBi + 2 zero-slope, balanced), the matching w_in column
slices (q/k/v/p) and w_out row slice. ReduceScatter(add) over each batch
group after out_proj, LN2 computed locally on each rank's 512-row shard.

Device dataflow is fully feature-major (zero on-device transposes):
  S^T[j,i] = matmul(lhsT=kT, rhs=qT); o^T = matmul(lhsT=v_tokmajor, rhs=E)
Softmax uses an analytic per-(q-tile,k-tile) shift (no max reduction): the
ALiBi mask slope*j minus shift slope*(i0+127) rides in the ACT exp bias; the
per-column residual cancels between numerator and denominator.
LN1 is folded into the qkvp matmul: x~ = x * rstd_bcast, g into w~, and the
(-rstd*mu)@u + 1@c correction rides as a K=2 extended contraction tile.
"""

import sys

sys.path.insert(0, "/opt/trn_rl_repo")

import numpy as np

import concourse.bass as bass
import concourse.mybir as mybir
import concourse.tile as tile
from concourse.bass_utils import run_bass_kernel_spmd

F32 = mybir.dt.float32
F32R = mybir.dt.float32r
BF16 = mybir.dt.bfloat16
NP_BF16 = mybir.dt.np(BF16)
AF = mybir.ActivationFunctionType
ALU = mybir.AluOpType

B, L, D, NHEADS, DH = 2, 2048, 1024, 16, 128
DEXP = 2048  # full d_expanded
NH = 4  # heads per core
DL = NH * DH  # 512, local d_expanded slice
KT = D // 128  # 8 k-tiles over d_model
NCH = L // 512  # 4 query chunks
NQT = L // 128  # 16 query tiles
NMT = L // 128  # 16 token tiles
NG = 4  # reduce-scatter groups (512 rows each)

# head assignment: rank r -> [sloped_windowed, sloped_full, zero, zero]
HGROUPS = [[0, 7, 8, 9], [1, 6, 10, 11], [2, 5, 12, 13], [3, 4, 14, 15]]
# per-slot block window (slot0 slopes >= 0.0924 -> 5 blocks is conservative)
WB = {0: 5, 1: 16, 2: 16, 3: 16}
SLOPED_SLOTS = (0, 1)

_CACHED = {}


def _normalize_waits(nc):
    """walrus wait-slot limits are tighter than what Tile emits for some
    instruction classes; move excess sync-waits onto same-engine Drain
    carriers inserted immediately before the instruction."""
    caps = {
        "InstDrain": 1,
        "InstDMACopy": 1,
        "InstCollectiveCompute": 1,
        "InstMemset": 1,
        "InstISA": 1,
        "InstTensorReduce": 1,
        "InstTensorTensor": 1,
        "InstTensorScalarPtr": 1,
        "InstTensorCopy": 1,
        "InstActivation": 1,
        "InstMatmult": 1,
        "InstBNStats": 1,
        "InstBNStatsAggregate": 1,
        "InstReciprocal": 1,
    }
    for func in nc.m.functions:
        for blk in func.blocks:
            insts = blk.instructions
            i = 0
            while i < len(insts):
                inst = insts[i]
                si = inst.sync_info
                cap = caps.get(type(inst).__name__, 1)
                if si is not None and len(si.on_wait or []) > cap:
                    waits = list(si.on_wait)
                    excess, keep = waits[:-cap], waits[-cap:]
                    for j, w in enumerate(excess):
                        d = mybir.InstNoOp(
                            name=f"{inst.name}-wsplit{j}",
                            engine=inst.engine,
                            ins=[],
                            outs=[],
                        )
                        d.sync_info = mybir.SyncInfo(on_wait=[w], on_update=[])
                        insts.insert(i, d)
                        nc.register_instruction(d, overwrite=True)
                        i += 1
                    si.on_wait = keep
                i += 1


def build(with_cc=True):
    nc = bass.Bass()

    xt_d = nc.dram_tensor("xt", [D, L], BF16, kind="ExternalInput")
    wq_d = nc.dram_tensor("wq", [D, DL], BF16, kind="ExternalInput")
    wk_d = nc.dram_tensor("wk", [D, DL], BF16, kind="ExternalInput")
    wv_d = nc.dram_tensor("wv", [D, DL], BF16, kind="ExternalInput")
    wp_d = nc.dram_tensor("wp", [D, DL], BF16, kind="ExternalInput")
    wout_d = nc.dram_tensor("wout", [DL, D], BF16, kind="ExternalInput")
    ln1g_d = nc.dram_tensor("ln1g", [D, 1], F32, kind="ExternalInput")
    ln1b_d = nc.dram_tensor("ln1b", [D, 1], F32, kind="ExternalInput")
    g2bc_d = nc.dram_tensor("g2bc", [128, D], F32, kind="ExternalInput")
    b2bc_d = nc.dram_tensor("b2bc", [128, D], F32, kind="ExternalInput")
    hbc_d = nc.dram_tensor("hbc", [128, 3 * NH], F32, kind="ExternalInput")
    biasv_d = nc.dram_tensor("biasv", [128, 24], F32, kind="ExternalInput")
    tri_d = nc.dram_tensor("tri", [128, 128], BF16, kind="ExternalInput")
    out_d = nc.dram_tensor("out", [NG * 128, D], F32, kind="ExternalOutput")

    with tile.TileContext(nc, pool_alloc_mode="queue") as tc:
        cp_cm = tc.tile_pool(name="const", bufs=1)
        cp = cp_cm.__enter__()

        # ---- tiny constants ----
        tri = cp.tile([128, 128], BF16, tag="tri")
        nc.sync.dma_start(tri[:], tri_d[:, :])
        hbc = cp.tile([128, 3 * NH], F32, tag="hbc")
        nc.sync.dma_start(hbc[:], hbc_d[:, :])
        biasv = cp.tile([128, 24], F32, tag="biasv")
        nc.sync.dma_start(biasv[:], biasv_d[:, :])

        wop_cm = tc.tile_pool(name="wo", bufs=1)
        wop = wop_cm.__enter__()
        woutT = []
        for h in range(NH):
            t = wop.tile([128, D], BF16, tag=f"woutT{h}", name=f"woutT{h}")
            woutT.append(t)
        g2bc = wop.tile([128, D], F32, tag="g2bc")
        b2bc = wop.tile([128, D], F32, tag="b2bc")

        ones_bf = cp.tile([128, 1], BF16, tag="ones_bf")
        nc.gpsimd.memset(ones_bf[:], 1.0)
        ones_row = cp.tile([1, 128], F32, tag="ones_row")
        nc.gpsimd.memset(ones_row[:], 1.0)
        eps1 = cp.tile([1, 1], F32, tag="eps1")
        nc.gpsimd.memset(eps1[:], 1e-5)
        eps128 = cp.tile([128, 1], F32, tag="eps128")
        nc.gpsimd.memset(eps128[:], 1e-5)

        inv_bc = [hbc[:, h : h + 1] for h in range(NH)]
        om_bc = [hbc[:, NH + h : NH + h + 1] for h in range(NH)]
        ratio_bc = [hbc[:, 2 * NH + h : 2 * NH + h + 1] for h in range(NH)]
        bias_v = {0: [biasv[:, d : d + 1] for d in range(WB[0])]}
        bias_w = [biasv[:, 5 + i : 5 + i + 1] for i in range(19)]  # slot1, idx = 4*ch-kb+3

        # ln1 per-partition columns [128, KT]
        g1c = cp.tile([128, KT], F32, tag="g1c")
        b1c = cp.tile([128, KT], F32, tag="b1c")
        nc.sync.dma_start(g1c[:], ln1g_d[:, :].rearrange("(a p) o -> p (a o)", p=128))
        nc.sync.dma_start(b1c[:], ln1b_d[:, :].rearrange("(a p) o -> p (a o)", p=128))
        g1bf = cp.tile([128, KT], BF16, tag="g1bf")
        b1bf = cp.tile([128, KT], BF16, tag="b1bf")
        nc.vector.tensor_copy(g1bf[:], g1c[:])
        nc.vector.tensor_copy(b1bf[:], b1c[:])

        # ---- stage 1: stats + raw-x bf16 cast in one pass ----
        resid_cm = tc.tile_pool(name="resid", bufs=1)
        resid = resid_cm.__enter__()  # geff + vtok

        dram_cm = tc.tile_pool(name="dram", bufs=1, space="DRAM")
        dram = dram_cm.__enter__()

        qkpA_cm = tc.tile_pool(name="qkpA", bufs=1)
        qkpA = qkpA_cm.__enter__()

        rowp_cm = tc.tile_pool(name="rows", bufs=1)
        rowp = rowp_cm.__enter__()  # rs_bc + xe + rs_cols, closed after stage 3

        xbp_cm = tc.tile_pool(name="xbp", bufs=1)
        xbp = xbp_cm.__enter__()
        wscp_cm = tc.tile_pool(name="wscp", bufs=1)
        wscp = wscp_cm.__enter__()

        xb = []
        wsc = {k: [] for k in ("q", "k", "v", "p")}
        we = {}
        wkinds = (("q", wq_d), ("k", wk_d), ("v", wv_d), ("p", wp_d))
        with (
            tc.tile_pool(name="xt_s", bufs=2) as xtp,
            tc.tile_pool(name="strow", bufs=1) as strp,
            tc.tile_pool(name="wraw", bufs=6) as wrp,
        ):
          with (
            tc.tile_pool(name="ps_stats", bufs=4, space="PSUM") as pstat,
            tc.tile_pool(name="ps_uc", bufs=4, space="PSUM") as puc,
          ):
            stats_ps = [pstat.tile([33, 512], F32, tag="stats", name=f"stats{i}") for i in range(NCH)]
            uc_ps = {k: puc.tile([33, 512], F32, tag="uc", name=f"uc{k}") for k, _ in wkinds}
            for kt in range(KT):
                xc = xbp.tile([128, L], BF16, tag=f"xb{kt}", name=f"xb{kt}")
                nc.sync.dma_start(xc[:], xt_d[kt * 128 : (kt + 1) * 128, :])
                xb.append(xc)
                xsq = xtp.tile([128, L], BF16, tag="xsq")
                nc.scalar.activation(xsq[:], xc[:], AF.Square)
                for ch in range(NCH):
                    sl = slice(ch * 512, (ch + 1) * 512)
                    nc.tensor.matmul(
                        stats_ps[ch][0:1, :], ones_bf[:], xc[:, sl],
                        start=(kt == 0), stop=(kt == KT - 1),
                    )
                    nc.tensor.matmul(
                        stats_ps[ch][32:33, :], ones_bf[:], xsq[:, sl],
                        start=(kt == 0), stop=(kt == KT - 1),
                    )
                for kind, wd in wkinds:
                    wr = wrp.tile([128, DL], BF16, tag="wr")
                    nc.sync.dma_start(wr[:], wd[kt * 128 : (kt + 1) * 128, :])
                    t = wscp.tile([128, DL], BF16, tag=f"w{kind}{kt}", name=f"w{kind}{kt}")
                    nc.vector.tensor_scalar_mul(t[:], wr[:], g1c[:, kt : kt + 1])
                    wsc[kind].append(t)
                    nc.tensor.matmul(
                        uc_ps[kind][0:1, :], g1bf[:, kt : kt + 1], wr[:],
                        start=(kt == 0), stop=(kt == KT - 1),
                    )
                    nc.tensor.matmul(
                        uc_ps[kind][32:33, :], b1bf[:, kt : kt + 1], wr[:],
                        start=(kt == 0), stop=(kt == KT - 1),
                    )
            for kind, _ in wkinds:
                wek = cp.tile([33, 512], BF16, tag=f"we{kind}", name=f"we{kind}")
                nc.gpsimd.memset(wek[:, :], 0.0)
                nc.scalar.copy(wek[0:1, :], uc_ps[kind][0:1, :])
                nc.scalar.copy(wek[32:33, :], uc_ps[kind][32:33, :])
                we[kind] = wek

            mu = strp.tile([1, L], F32, tag="mu")
            msq = strp.tile([1, L], F32, tag="msq")
            for ch in range(NCH):
                sl = slice(ch * 512, (ch + 1) * 512)
                nc.vector.tensor_scalar_mul(mu[:, sl], stats_ps[ch][0:1, :], 1.0 / D)
                nc.vector.tensor_scalar_mul(msq[:, sl], stats_ps[ch][32:33, :], 1.0 / D)
          if True:

            sd = strp.tile([1, L], F32, tag="rtmp2")
            nc.vector.tensor_mul(sd[:], mu[:], mu[:])
            nc.vector.tensor_sub(msq[:], msq[:], sd[:])  # msq now holds var
            nc.scalar.activation(sd[:], msq[:], AF.Sqrt, bias=eps1[:])
            rsd = strp.tile([1, L], F32, tag="rsd")
            nc.vector.reciprocal(rsd[:], sd[:])
            rs_bc = rowp.tile([128, L], F32, tag="rs_bc")
            with tc.tile_pool(name="ps_bc", bufs=4, space="PSUM") as pbc:
                for ch in range(NCH):
                    sl = slice(ch * 512, (ch + 1) * 512)
                    bc_ps = pbc.tile([128, 512], F32, tag="bcps", name=f"bcps{ch}")
                    nc.tensor.matmul(bc_ps[:], ones_row[:], rsd[:, sl], start=True, stop=True)
                    nc.scalar.copy(rs_bc[:, sl], bc_ps[:])
            # per-token rstd in column layout [128, 16] via DRAM roundtrip
            rs_scr = dram.tile([L, 1], F32, tag="rs_scr")
            nc.sync.dma_start(rs_scr[:, :], rsd[:, :])
            rs_cols = rowp.tile([128, NMT], F32, tag="rs_cols")
            nc.sync.dma_start(
                rs_cols[:], rs_scr[:, :].rearrange("(a p) o -> p (a o)", p=128)
            )
            # extended contraction rows: row0 = -mu, row32 = sd (=1/rstd), rest 0
            xe = rowp.tile([33, L], BF16, tag="xe")
            nc.gpsimd.memset(xe[:, :], 0.0)
            nc.vector.tensor_scalar_mul(xe[0:1, :], mu[:], -1.0)
            nc.vector.tensor_copy(xe[32:33, :], sd[:])

        # ---- stages 3-5: head-pipelined qkvp + attention; out_proj rides h3 ----
        geff = [resid.tile([128, L], BF16, tag=f"geff{h}", name=f"geff{h}") for h in range(NH)]
        vtok = []
        qT = [qkpA.tile([128, L], BF16, tag=f"qT{h}", name=f"qT{h}") for h in range(NH)]
        kS = [qkpA.tile([128, L], BF16, tag=f"kS{h}", name=f"kS{h}") for h in range(NH)]
        rs_in = [dram.tile([512, D], F32, tag=f"rsin{g}", name=f"rsin{g}") for g in range(NG)]
        rs_out = [dram.tile([128, D], F32, tag=f"rsout{g}", name=f"rsout{g}") for g in range(NG)]

        pmm_cm = tc.tile_pool(name="ps_mm", bufs=2, space="PSUM")
        pmm = pmm_cm.__enter__()
        kkp_cm = tc.tile_pool(name="kk", bufs=1)
        kkp = kkp_cm.__enter__()
        pscrp_cm = tc.tile_pool(name="pscrp", bufs=2)
        pscrp = pscrp_cm.__enter__()
        with (
            tc.tile_pool(name="ps_s", bufs=2, space="PSUM") as pss,
            tc.tile_pool(name="ps_o", bufs=2, space="PSUM") as pso,
            tc.tile_pool(name="ps_den", bufs=1, space="PSUM") as psd,
            tc.tile_pool(name="et", bufs=6) as etp,
            tc.tile_pool(name="dn", bufs=3) as dnp,
            tc.tile_pool(name="ln2", bufs=1) as lnp,
            tc.tile_pool(name="ostage", bufs=2) as osp,
        ):

            def emit_v():
                for m in range(NMT):
                    msl = slice(m * 128, (m + 1) * 128)
                    vps = pmm.tile([128, 512], F32, tag="mm", name=f"vps{m}")
                    for kt in range(KT):
                        nc.tensor.matmul(vps[:], xb[kt][:, msl], wsc["v"][kt][:],
                                         start=(kt == 0), stop=False)
                    nc.tensor.matmul(vps[:], xe[:, msl], we["v"][:], start=False, stop=True)
                    vt = resid.tile([128, DL], BF16, tag=f"vtok{m}", name=f"vtok{m}")
                    nc.scalar.activation(vt[:], vps[:], AF.Copy, scale=rs_cols[:, m : m + 1])
                    vtok.append(vt)

            def emit_qkvp(h):
                hsl = slice(h * 128, (h + 1) * 128)
                kk = kkp.tile([128, L], F32, tag="kk", name=f"kk{h}")
                for ch in range(NCH):
                    csl = slice(ch * 512, (ch + 1) * 512)
                    qps = pmm.tile([128, 512], F32, tag="mm", name=f"qps{h}_{ch}")
                    for kt in range(KT):
                        nc.tensor.matmul(qps[:], wsc["q"][kt][:, hsl], xb[kt][:, csl],
                                         start=(kt == 0), stop=False)
                    nc.tensor.matmul(qps[:], we["q"][:, hsl], xe[:, csl],
                                     start=False, stop=True)
                    nc.vector.scalar_tensor_tensor(
                        qT[h][:, csl], qps[:], inv_bc[h], rs_bc[:, csl], ALU.mult, ALU.mult
                    )

                    kps = pmm.tile([128, 512], F32, tag="mm", name=f"kps{h}_{ch}")
                    for kt in range(KT):
                        nc.tensor.matmul(kps[:], wsc["k"][kt][:, hsl], xb[kt][:, csl],
                                         start=(kt == 0), stop=False)
                    nc.tensor.matmul(kps[:], we["k"][:, hsl], xe[:, csl],
                                     start=False, stop=True)
                    nc.vector.scalar_tensor_tensor(
                        kk[:, csl], kps[:], om_bc[h], rs_bc[:, csl], ALU.mult, ALU.mult
                    )

                    pps = pmm.tile([128, 512], F32, tag="mm", name=f"pps{h}_{ch}")
                    for kt in range(KT):
                        nc.tensor.matmul(pps[:], wsc["p"][kt][:, hsl], xb[kt][:, csl],
                                         start=(kt == 0), stop=False)
                    nc.tensor.matmul(pps[:], we["p"][:, hsl], xe[:, csl],
                                     start=False, stop=True)
                    pscr = pscrp.tile([128, 512], F32, tag="pscr")
                    nc.vector.tensor_mul(pscr[:], pps[:], rs_bc[:, csl])
                    nc.scalar.activation(geff[h][:, csl], pscr[:], AF.Silu)
                # smear
                nc.vector.scalar_tensor_tensor(
                    kS[h][:, 1:L], kk[:, 0 : L - 1], ratio_bc[h], kk[:, 1:L],
                    ALU.mult, ALU.add,
                )
                nc.vector.tensor_copy(kS[h][:, 0:1], kk[:, 0:1])

            def emit_attn(h, chs=None):
                hsl = slice(h * 128, (h + 1) * 128)
                for ch in (range(NCH) if chs is None else chs):
                    csl = slice(ch * 512, (ch + 1) * 512)
                    kb_lo = max(0, 4 * ch + 1 - WB[h])
                    kb_hi = 4 * ch + 3
                    ops_ps = pso.tile([128, 512], F32, tag="ops", name=f"ops{h}_{ch}")
                    den_ps = psd.tile([1, 512], F32, tag="den", name=f"den{h}_{ch}")
                    first = {qs: None for qs in range(4)}
                    for kb in range(kb_lo, kb_hi + 1):
                        # valid query subtiles for this key block
                        qs0 = max(0, kb - 4 * ch)
                        qs1 = min(4, kb - 4 * ch + WB[h])
                        if qs0 >= qs1:
                            continue
                        nsl = slice(csl.start + qs0 * 128, csl.start + qs1 * 128)
                        esl = slice(qs0 * 128, qs1 * 128)
                        sps = pss.tile([128, 512], F32, tag="sps", name=f"sps{h}_{ch}_{kb}")
                        nc.tensor.matmul(
                            sps[:, esl], kS[h][:, kb * 128 : (kb + 1) * 128],
                            qT[h][:, nsl], start=True, stop=True,
                        )
                        et = etp.tile([128, 512], BF16, tag="et")
                        if h == 0:
                            for qs in range(qs0, qs1):
                                qsl = slice(qs * 128, (qs + 1) * 128)
                                dd = (4 * ch + qs) - kb
                                nc.scalar.activation(
                                    et[:, qsl], sps[:, qsl], AF.Exp,
                                    bias=bias_v[h][dd],
                                )
                        elif h == 1:
                            nc.scalar.activation(
                                et[:, esl], sps[:, esl], AF.Exp,
                                bias=bias_w[4 * ch - kb + 3],
                            )
                        else:
                            nc.scalar.activation(et[:, esl], sps[:, esl], AF.Exp)
                        for qs in range(qs0, qs1):
                            if (4 * ch + qs) == kb:
                                qsl = slice(qs * 128, (qs + 1) * 128)
                                nc.vector.tensor_mul(et[:, qsl], et[:, qsl], tri[:])
                        st = all(first[qs] is None for qs in range(qs0, qs1))
                        for qs in range(qs0, qs1):
                            if first[qs] is None:
                                first[qs] = kb
                        nc.tensor.matmul(
                            ops_ps[:, esl], vtok[kb][:, hsl], et[:, esl],
                            start=st, stop=(kb == kb_hi),
                        )
                        nc.tensor.matmul(
                            den_ps[:, esl], ones_bf[:], et[:, esl],
                            start=st, stop=(kb == kb_hi),
                        )
                    dinv = dnp.tile([1, 512], BF16, tag="dinv")
                    with nc.allow_low_precision("bf16 1/den feeds a bf16 matmul"):
                        nc.vector.reciprocal(dinv[:], den_ps[:])
                    dbc_ps = psd.tile([128, 512], F32, tag="dbcps", name=f"dbcps{h}_{ch}")
                    nc.tensor.matmul(dbc_ps[:], ones_bfr[:], dinv[:], start=True, stop=True)
                    dbc = dnp.tile([128, 512], F32, tag="dbc")
                    nc.vector.tensor_copy(dbc[:], dbc_ps[:])
                    ozc = dnp.tile([128, 512], BF16, tag="ozc")
                    nc.vector.tensor_mul(ozc[:], ops_ps[:], dbc[:])
                    nc.vector.tensor_mul(geff[h][:, csl], ozc[:], geff[h][:, csl])

                    if h == NH - 1:
                        g = ch
                        for mi in range(4):
                            m = 4 * g + mi
                            msl = slice(m * 128, (m + 1) * 128)
                            for nch2 in range(2):
                                nsl2 = slice(nch2 * 512, (nch2 + 1) * 512)
                                op2 = pmm.tile([128, 512], F32, tag="mm", name=f"op2_{m}_{nch2}")
                                for hh in range(NH):
                                    nc.tensor.matmul(
                                        op2[:], geff[hh][:, msl], woutT[hh][:, nsl2],
                                        start=(hh == 0), stop=(hh == NH - 1),
                                    )
                                osb = osp.tile([128, 512], F32, tag="osb")
                                nc.vector.tensor_copy(osb[:], op2[:])
                                nc.sync.dma_start(
                                    rs_in[g][mi * 128 : (mi + 1) * 128, nsl2], osb[:]
                                )
                        if with_cc:
                            nc.gpsimd.collective_compute(
                                "ReduceScatter", ALU.add,
                                replica_groups=[[0, 1, 2, 3], [4, 5, 6, 7]],
                                ins=[rs_in[g][:, :].opt()],
                                outs=[rs_out[g][:, :].opt()],
                            )
                        else:
                            nc.sync.dma_start(rs_out[g][:, :], rs_in[g][0:128, :])
                        yt = lnp.tile([128, D], F32, tag="yt")
                        nc.sync.dma_start(yt[:], rs_out[g][:, :])
                        bs = lnp.tile([128, 12], F32, tag="bs")
                        nc.vector.bn_stats(bs[:, 0:6], yt[:, 0:512])
                        nc.vector.bn_stats(bs[:, 6:12], yt[:, 512:1024])
                        ag = lnp.tile([128, 2], F32, tag="ag")
                        nc.vector.bn_aggr(ag[:], bs[:])
                        sd2 = lnp.tile([128, 1], F32, tag="sd2")
                        nc.scalar.activation(sd2[:], ag[:, 1:2], AF.Sqrt, bias=eps128[:])
                        rstd2 = lnp.tile([128, 1], F32, tag="rstd2")
                        nc.vector.reciprocal(rstd2[:], sd2[:])
                        nmu = lnp.tile([128, 1], F32, tag="nmu")
                        nc.vector.scalar_tensor_tensor(
                            nmu[:], ag[:, 0:1], -1.0, rstd2[:], ALU.mult, ALU.mult
                        )
                        t2 = lnp.tile([128, D], F32, tag="t2")
                        nc.scalar.activation(t2[:], yt[:], AF.Identity, bias=nmu[:], scale=rstd2[:])
                        t3 = lnp.tile([128, D], F32, tag="t3")
                        nc.vector.tensor_mul(t3[:], t2[:], g2bc[:])
                        nc.vector.tensor_add(t3[:], t3[:], b2bc[:])
                        nc.sync.dma_start(out_d[g * 128 : (g + 1) * 128, :], t3[:])

            ones_bfr = cp.tile([1, 128], BF16, tag="ones_bfr")
            nc.gpsimd.memset(ones_bfr[:], 1.0)

            # software-pipelined emission: attention(h) interleaves with qkvp(h+1)
            emit_v()
            emit_qkvp(0)
            emit_qkvp(1)
            emit_attn(0)
            emit_qkvp(2)
            emit_attn(1)
            emit_qkvp(3)
            for h in range(NH):
                nc.sync.dma_start(woutT[h][:], wout_d[h * 128 : (h + 1) * 128, :])
            nc.sync.dma_start(g2bc[:], g2bc_d[:, :])
            nc.sync.dma_start(b2bc[:], b2bc_d[:, :])
            for ch in range(NCH):
                emit_attn(2, [ch])
                emit_attn(3, [ch])

        pscrp_cm.__exit__(None, None, None)
        kkp_cm.__exit__(None, None, None)
        pmm_cm.__exit__(None, None, None)
        wscp_cm.__exit__(None, None, None)
        xbp_cm.__exit__(None, None, None)
        rowp_cm.__exit__(None, None, None)
        qkpA_cm.__exit__(None, None, None)
        dram_cm.__exit__(None, None, None)
        resid_cm.__exit__(None, None, None)
        wop_cm.__exit__(None, None, None)
        cp_cm.__exit__(None, None, None)

    _normalize_waits(nc)
    return nc


def _slopes16():
    half = NHEADS // 2
    return np.concatenate(
        [2.0 ** np.linspace(0.0, -8.0, half), np.zeros(NHEADS - half)]
    ).astype(np.float32)


def kernel(x, ln1_g, ln1_b, ln2_g, ln2_b, w_in, w_out, smear_factor, log_scale):
    x = np.asarray(x, np.float32)
    w_in = np.asarray(w_in, np.float32)
    w_out = np.asarray(w_out, np.float32)
    ln1_g = np.asarray(ln1_g, np.float32)
    ln1_b = np.asarray(ln1_b, np.float32)
    ln2_g = np.asarray(ln2_g, np.float32)
    ln2_b = np.asarray(ln2_b, np.float32)
    smear_factor = np.asarray(smear_factor, np.float32)
    log_scale = np.asarray(log_scale, np.float32)

    if "nc" not in _CACHED:
        _CACHED["nc"] = build()
    nc = _CACHED["nc"]

    slopes16 = _slopes16()
    jj = np.arange(128)
    tri = (jj[:, None] <= jj[None, :]).astype(NP_BF16)  # keep j <= i

    in_maps = []
    for c in range(8):
        b, r = divmod(c, 4)
        hs = HGROUPS[r]
        cols = np.concatenate([np.arange(h * 128, (h + 1) * 128) for h in hs])
        sl = slopes16[hs]
        inv = np.exp(-2.0 * log_scale[hs]) / np.sqrt(128.0)
        sg = 1.0 / (1.0 + np.exp(-smear_factor[hs]))
        om = 1.0 - sg
        ratio = np.exp(smear_factor[hs])
        hbc = np.tile(
            np.concatenate([inv, om, ratio]).reshape(1, 3 * NH), (128, 1)
        ).astype(np.float32)
        iota_c = np.arange(128, dtype=np.float32)
        bias_cols = [sl[0] * (iota_c - 128 * d - 63) for d in range(WB[0])]
        # slot1: one vector per dd = 4*ch - kb in [-3, 15]:
        # bias = slope*(j_loc + 128*kb - 512*ch - 447) = slope*(j_loc - 128*dd - 447)
        bias_cols += [sl[1] * (iota_c - 128 * d - 447) for d in range(-3, 16)]
        biasv = np.stack(bias_cols, axis=1).astype(np.float32)
        m = {
            "xt": np.ascontiguousarray(x[b].T).astype(NP_BF16),
            "wq": np.ascontiguousarray(w_in[:, 0 * DEXP + cols]).astype(NP_BF16),
            "wk": np.ascontiguousarray(w_in[:, 1 * DEXP + cols]).astype(NP_BF16),
            "wv": np.ascontiguousarray(w_in[:, 2 * DEXP + cols]).astype(NP_BF16),
            "wp": np.ascontiguousarray(w_in[:, 3 * DEXP + cols]).astype(NP_BF16),
            "wout": np.ascontiguousarray(w_out[cols, :]).astype(NP_BF16),
            "ln1g": ln1_g.reshape(D, 1),
            "ln1b": ln1_b.reshape(D, 1),
            "g2bc": np.tile(ln2_g.reshape(1, D), (128, 1)),
            "b2bc": np.tile(ln2_b.reshape(1, D), (128, 1)),
            "hbc": hbc,
            "biasv": biasv,
            "tri": tri,
        }
        in_maps.append(m)

    res = None
    last_exc = None
    for _attempt in range(3):
        try:
            res = run_bass_kernel_spmd(nc, in_maps, core_ids=list(range(8)))
            break
        except Exception as e:  # transient axon worker drops; retry
            last_exc = e
            import time as _time

            _time.sleep(2.0)
    if res is None:
        raise last_exc
    _CACHED["last_res"] = res
    out = np.empty((B, L, D), np.float32)
    for c in range(8):
        b, r = divmod(c, 4)
        o = res.results[c]["out"]  # [512, 1024]
        for g in range(NG):
            out[b, 512 * g + 128 * r : 512 * g + 128 * r + 128, :] = o[
                128 * g : 128 * (g + 1), :
            ]
    return out



# revision 32
# speedup vs baseline: 1.3089x; 1.3089x over previous
"""Trainium2 Bass kernel for nn_Block_65755949302136 (dense transformer block).

Sharding: 8 cores = 2 (batch) x 4 (tensor-parallel ranks). Each rank owns 4
heads (2 sloped-ALiBi + 2 zero-slope, balanced), the matching w_in column
slices (q/k/v/p) and w_out row slice. ReduceScatter(add) over each batch
group after out_proj, LN2 computed locally on each rank's 512-row shard.

v2 dataflow (all feature-major, no on-device transposes of activations):
  - LN1 gamma folded into W host-side; beta rides the ACT bias slots.
  - x is centered+normalized ONCE into xn (bf16) via two DVE tensor-tensor
    ops against PE-broadcast rstd rows; q/k/p GEMMs read xn, so no extended
    contraction tile and no per-output rstd multiply.
  - v GEMM runs on RAW x during the centering window (keeps PE busy);
    its -mu*colsum(Wv) correction is a token-major DVE stt against a
    host-sent colsum broadcast, scaled by rstd in the ACT copy.
  - Softmax denominator: ap=1 matmuls (free on PE) accumulate per-query
    den columns in PSUM; one PE transpose + reciprocal + 4 outer-product
    matmuls rebuild the [dh, q] reciprocal broadcast.
  - Slot-0 (steep-slope) heads use a 2-block attention window.
"""

import sys

sys.path.insert(0, "/opt/trn_rl_repo")

import numpy as np

import concourse.bass as bass
import concourse.mybir as mybir
import concourse.tile as tile
from concourse.bass_utils import run_bass_kernel_spmd

F32 = mybir.dt.float32
F32R = mybir.dt.float32r
BF16 = mybir.dt.bfloat16
NP_BF16 = mybir.dt.np(BF16)
AF = mybir.ActivationFunctionType
ALU = mybir.AluOpType

B, L, D, NHEADS, DH = 2, 2048, 1024, 16, 128
DEXP = 2048  # full d_expanded
NH = 4  # heads per core
DL = NH * DH  # 512, local d_expanded slice
KT = D // 128  # 8 k-tiles over d_model
NCH = L // 512  # 4 query chunks
NQT = L // 128  # 16 query tiles
NMT = L // 128  # 16 token tiles
NG = 4  # reduce-scatter groups (512 rows each)

# head assignment: rank r -> [sloped_windowed, sloped_full, zero, zero]
HGROUPS = [[0, 7, 8, 9], [1, 6, 10, 11], [2, 5, 12, 13], [3, 4, 14, 15]]
# per-slot block window (slot0 slopes >= 0.0924: dropped mass < 1e-4 at WB=2)
WB = {0: 2, 1: 16, 2: 16, 3: 16}
NB0 = WB[0]  # slot-0 bias columns in biasv

_CACHED = {}


def _normalize_waits(nc):
    """walrus wait-slot limits are tighter than what Tile emits for some
    instruction classes; move excess sync-waits onto same-engine Drain
    carriers inserted immediately before the instruction."""
    for func in nc.m.functions:
        for blk in func.blocks:
            insts = blk.instructions
            i = 0
            while i < len(insts):
                inst = insts[i]
                si = inst.sync_info
                cap = 1
                if si is not None and len(si.on_wait or []) > cap:
                    waits = list(si.on_wait)
                    excess, keep = waits[:-cap], waits[-cap:]
                    for j, w in enumerate(excess):
                        d = mybir.InstNoOp(
                            name=f"{inst.name}-wsplit{j}",
                            engine=inst.engine,
                            ins=[],
                            outs=[],
                        )
                        d.sync_info = mybir.SyncInfo(on_wait=[w], on_update=[])
                        insts.insert(i, d)
                        nc.register_instruction(d, overwrite=True)
                        i += 1
                    si.on_wait = keep
                i += 1


def build(with_cc=True, b1_zero=True, ln2_trivial=True):
    nc = bass.Bass()

    xt_d = nc.dram_tensor("xt", [D, L], BF16, kind="ExternalInput")
    wq_d = nc.dram_tensor("wq", [D, DL], BF16, kind="ExternalInput")
    wk_d = nc.dram_tensor("wk", [D, DL], BF16, kind="ExternalInput")
    wv_d = nc.dram_tensor("wv", [D, DL], BF16, kind="ExternalInput")
    wp_d = nc.dram_tensor("wp", [D, DL], BF16, kind="ExternalInput")
    wout_d = nc.dram_tensor("wout", [DL, D], BF16, kind="ExternalInput")
    hbc_d = nc.dram_tensor("hbc", [128, 3 * NH], F32, kind="ExternalInput")
    qkb_d = nc.dram_tensor("qkb", [128, 3 * NH], F32, kind="ExternalInput")
    vcb_d = nc.dram_tensor("vcb", [128, DL], F32, kind="ExternalInput")
    biasv_d = nc.dram_tensor("biasv", [128, NB0 + 19], F32, kind="ExternalInput")
    tri_d = nc.dram_tensor("tri", [128, 128], BF16, kind="ExternalInput")
    idn_d = nc.dram_tensor("idn", [128, 128], F32, kind="ExternalInput")
    idnb_d = nc.dram_tensor("idnb", [128, 128], BF16, kind="ExternalInput")
    if not b1_zero:
        bvbc_d = nc.dram_tensor("bvbc", [128, DL], F32, kind="ExternalInput")
    if not ln2_trivial:
        g2bc_d = nc.dram_tensor("g2bc", [128, D], F32, kind="ExternalInput")
        b2bc_d = nc.dram_tensor("b2bc", [128, D], F32, kind="ExternalInput")
    out_d = nc.dram_tensor("out", [NG * 128, D], F32, kind="ExternalOutput")

    with tile.TileContext(nc, pool_alloc_mode="queue") as tc:
        cp_cm = tc.tile_pool(name="const", bufs=1)
        cp = cp_cm.__enter__()

        # ---- tiny constants (DMAs are emitted after the x/wv loads: the
        # sync queue is serial and x gates the whole front of the kernel) ----
        tri = cp.tile([128, 128], BF16, tag="tri")
        idn = cp.tile([128, 128], F32, tag="idn")
        idnb = cp.tile([128, 128], BF16, tag="idnb")
        hbc = cp.tile([128, 3 * NH], F32, tag="hbc")
        qkb = cp.tile([128, 3 * NH], F32, tag="qkb")
        vcb = cp.tile([128, DL], F32, tag="vcb")
        biasv = cp.tile([128, NB0 + 19], F32, tag="biasv")

        def emit_const_dmas():
            nc.sync.dma_start(idnb[:], idnb_d[:, :])
            nc.sync.dma_start(vcb[:], vcb_d[:, :])
            nc.sync.dma_start(hbc[:], hbc_d[:, :])
            nc.sync.dma_start(qkb[:], qkb_d[:, :])
            nc.sync.dma_start(tri[:], tri_d[:, :])
            nc.sync.dma_start(idn[:], idn_d[:, :])
            nc.sync.dma_start(biasv[:], biasv_d[:, :])

        ones_bf = cp.tile([128, 1], BF16, tag="ones_bf")
        nc.gpsimd.memset(ones_bf[:], 1.0)
        ones_bfr = cp.tile([1, 128], BF16, tag="ones_bfr")
        nc.gpsimd.memset(ones_bfr[:], 1.0)
        ones128b = cp.tile([128, 128], BF16, tag="ones128b")
        nc.gpsimd.memset(ones128b[:], 1.0)
        eps128 = cp.tile([128, 1], F32, tag="eps128")
        nc.gpsimd.memset(eps128[:], 1e-5)

        inv_bc = [hbc[:, h : h + 1] for h in range(NH)]
        om_bc = [hbc[:, NH + h : NH + h + 1] for h in range(NH)]
        ratio_bc = [hbc[:, 2 * NH + h : 2 * NH + h + 1] for h in range(NH)]
        bqi = [qkb[:, h : h + 1] for h in range(NH)]
        bko = [qkb[:, NH + h : NH + h + 1] for h in range(NH)]
        bp = [qkb[:, 2 * NH + h : 2 * NH + h + 1] for h in range(NH)]
        bias_v = {0: [biasv[:, d : d + 1] for d in range(NB0)]}
        bias_w = [biasv[:, NB0 + i : NB0 + i + 1] for i in range(19)]

        resid_cm = tc.tile_pool(name="resid", bufs=1)
        resid = resid_cm.__enter__()  # geff + vtok
        dram_cm = tc.tile_pool(name="dram", bufs=1, space="DRAM")
        dram = dram_cm.__enter__()
        qkpA_cm = tc.tile_pool(name="qkpA", bufs=1)
        qkpA = qkpA_cm.__enter__()
        rowp_cm = tc.tile_pool(name="rows", bufs=1)
        rowp = rowp_cm.__enter__()  # rs_bc, mrs_bc, nmu/rs cols, row scratch
        xnp_cm = tc.tile_pool(name="xnp", bufs=1)
        xnp = xnp_cm.__enter__()
        wscp_cm = tc.tile_pool(name="wscp", bufs=1)
        wscp = wscp_cm.__enter__()

        xbp_cm = tc.tile_pool(name="xbp", bufs=1)
        xbp = xbp_cm.__enter__()

        xb = []
        wsc = {}
        wkinds = (("v", wv_d), ("q", wq_d), ("k", wk_d), ("p", wp_d))
        geff = [resid.tile([128, L], BF16, tag=f"geff{h}", name=f"geff{h}") for h in range(NH)]
        vtok = []
        qT = [qkpA.tile([128, L], BF16, tag=f"qT{h}", name=f"qT{h}") for h in range(NH)]
        kS = [qkpA.tile([128, L], BF16, tag=f"kS{h}", name=f"kS{h}") for h in range(NH)]
        xn = [xnp.tile([128, L], BF16, tag=f"xn{kt}", name=f"xn{kt}") for kt in range(KT)]
        rs_in = [dram.tile([512, D], F32, tag=f"rsin{g}", name=f"rsin{g}") for g in range(NG)]
        rs_out = [dram.tile([128, D], F32, tag=f"rsout{g}", name=f"rsout{g}") for g in range(NG)]

        rs_bc = rowp.tile([128, L], BF16, tag="rs_bc")
        mrs_bc = rowp.tile([128, L], BF16, tag="mrs_bc")
        # token-major stat columns [128, 16]
        ncol = rowp.tile([128, NMT], F32, tag="ncol")
        sqm = rowp.tile([128, NMT], F32, tag="sqm")
        rs_cols = rowp.tile([128, NMT], F32, tag="rs_cols")
        nmrsc = rowp.tile([128, NMT], F32, tag="nmrsc")
        nmu_cols = ncol

        if not b1_zero:
            bvbc = cp.tile([128, DL], F32, tag="bvbc")
            nc.sync.dma_start(bvbc[:], bvbc_d[:, :])

        # ---- stage A: x/wv DMA + column stats (ap=1 matmuls, ~free on PE)
        # + first v-chain group kt-major to fill PE during the DMA window ----
        NG1 = 6
        vmm_cm = tc.tile_pool(name="ps_vm", bufs=NG1, space="PSUM")
        vmm = vmm_cm.__enter__()
        vps_g1 = [vmm.tile([128, 512], F32, tag="vmm", name=f"vps{m}") for m in range(NG1)]
        with (
            tc.tile_pool(name="ps_sc", bufs=1, space="PSUM") as pscol,
        ):
            scol = pscol.tile([128, NMT], F32, tag="scol", name="scol")
            sqcol = pscol.tile([128, NMT], F32, tag="sqcol", name="sqcol")
            for kt in range(KT):
                xc = xbp.tile([128, L], BF16, tag=f"xb{kt}", name=f"xb{kt}")
                nc.sync.dma_start(xc[:], xt_d[kt * 128 : (kt + 1) * 128, :])
                xb.append(xc)
                wvt = wscp.tile([128, DL], BF16, tag=f"wv{kt}", name=f"wv{kt}")
                nc.sync.dma_start(wvt[:], wv_d[kt * 128 : (kt + 1) * 128, :])
                wsc.setdefault("v", []).append(wvt)
                xsq = xn[kt]  # xn doubles as the x^2 staging before centering
                nc.scalar.activation(xsq[:], xc[:], AF.Square)
                # NOTE: start=True resets the WHOLE psum bank, so only the
                # very first matmul touching each bank may carry it.
                for m in range(NMT):
                    msl = slice(m * 128, (m + 1) * 128)
                    nc.tensor.matmul(
                        scol[:, m : m + 1], xc[:, msl], ones_bf[:],
                        start=(kt == 0 and m == 0), stop=(kt == KT - 1),
                    )
                    nc.tensor.matmul(
                        sqcol[:, m : m + 1], xsq[:, msl], ones_bf[:],
                        start=(kt == 0 and m == 0), stop=(kt == KT - 1),
                    )
                for m in range(NG1):
                    nc.tensor.matmul(
                        vps_g1[m][:], xc[:, m * 128 : (m + 1) * 128], wvt[:],
                        start=(kt == 0), stop=(kt == KT - 1),
                    )
            emit_const_dmas()
            for kind, wd in (("q", wq_d), ("k", wk_d), ("p", wp_d)):
                tiles = []
                for kt in range(KT):
                    t = wscp.tile([128, DL], BF16, tag=f"w{kind}{kt}", name=f"w{kind}{kt}")
                    nc.sync.dma_start(t[:], wd[kt * 128 : (kt + 1) * 128, :])
                    tiles.append(t)
                wsc[kind] = tiles

            # column-space LN1 stats: all ops on [128, 16]
            nc.vector.tensor_scalar_mul(ncol[:], scol[:], -1.0 / D)
            nc.vector.tensor_scalar_mul(sqm[:], sqcol[:], 1.0 / D)
            nc.vector.tensor_mul(nmrsc[:], ncol[:], ncol[:])
            nc.vector.tensor_sub(sqm[:], sqm[:], nmrsc[:])  # var
            nc.scalar.activation(sqm[:], sqm[:], AF.Sqrt, bias=eps128[:])
            nc.vector.reciprocal(rs_cols[:], sqm[:])
            nc.vector.tensor_mul(nmrsc[:], ncol[:], rs_cols[:])

        # ---- stage B: broadcasts, remaining v chains, centering ----
        def emit_vpost(m, vps):
            # v += (-mu) * colsum(Wv) (token-major), then *rstd in the copy
            nc.vector.scalar_tensor_tensor(
                vps[:], vcb[:], nmu_cols[:, m : m + 1], vps[:], ALU.mult, ALU.add
            )
            if not b1_zero:
                nc.vector.tensor_add(vps[:], vps[:], bvbc[:])
            vt = resid.tile([128, DL], BF16, tag=f"vtok{m}", name=f"vtok{m}")
            nc.scalar.activation(vt[:], vps[:], AF.Copy, scale=rs_cols[:, m : m + 1])
            vtok.append(vt)

        def emit_vchain(m):
            msl = slice(m * 128, (m + 1) * 128)
            vps = vmm.tile([128, 512], F32, tag="vmm", name=f"vps{m}")
            for kt in range(KT):
                nc.tensor.matmul(vps[:], xb[kt][:, msl], wsc["v"][kt][:],
                                 start=(kt == 0), stop=(kt == KT - 1))
            return vps

        # free three G1 slots, start two G2 chains to cover the broadcast wait
        for m in range(3):
            emit_vpost(m, vps_g1[m])
        vps_pend = [(6, emit_vchain(6)), (7, emit_vchain(7))]

        # rstd / (-mu*rstd) broadcasts: diag(cols) matmul against all-ones
        with (
            tc.tile_pool(name="pbc", bufs=2, space="PSUM") as pbc,
            tc.tile_pool(name="dgp", bufs=4) as dgp,
        ):
            for si, (src, dst) in enumerate(((rs_cols, rs_bc), (nmrsc, mrs_bc))):
                for ch in range(NCH):
                    sl = slice(ch * 512, (ch + 1) * 512)
                    bps = pbc.tile([128, 512], F32, tag="bcps", name=f"bc{si}_{ch}")
                    for mi in range(4):
                        m = 4 * ch + mi
                        dg = dgp.tile([128, 128], BF16, tag="dg")
                        nc.vector.tensor_scalar_mul(dg[:], idnb[:], src[:, m : m + 1])
                        nc.tensor.matmul(bps[:, mi * 128 : (mi + 1) * 128],
                                         ones128b[:], dg[:], start=(mi == 0),
                                         stop=(mi == 3), skip_group_check=True)
                    nc.scalar.copy(dst[:, sl], bps[:])

        for m in range(3, NG1):
            emit_vpost(m, vps_g1[m])
        for m in range(8, NMT):
            vps_pend.append((m, emit_vchain(m)))
            if len(vps_pend) >= 3:
                emit_vpost(*vps_pend.pop(0))
        while vps_pend:
            emit_vpost(*vps_pend.pop(0))

        # centering: xn = x * rs_bc, then += mrs_bc in place (bf16, 2x DVE)
        # per 512-chunk so qkp chains unlock chunk-by-chunk behind it
        for ch in range(NCH):
            csl = slice(ch * 512, (ch + 1) * 512)
            for kt in range(KT):
                nc.vector.tensor_mul(xn[kt][:, csl], xb[kt][:, csl], rs_bc[:, csl])
                nc.vector.tensor_add(xn[kt][:, csl], xn[kt][:, csl], mrs_bc[:, csl])

        vmm_cm.__exit__(None, None, None)
        xbp_cm.__exit__(None, None, None)  # raw x no longer needed
        pmm_cm = tc.tile_pool(name="ps_mm", bufs=2, space="PSUM")
        pmm = pmm_cm.__enter__()

        wop_cm = tc.tile_pool(name="wo", bufs=1)
        wop = wop_cm.__enter__()
        woutT = []
        for h in range(NH):
            t = wop.tile([128, D], BF16, tag=f"woutT{h}", name=f"woutT{h}")
            woutT.append(t)
        if not ln2_trivial:
            g2bc = wop.tile([128, D], F32, tag="g2bc")
            b2bc = wop.tile([128, D], F32, tag="b2bc")

        # ---- stages C-E: head-pipelined qkp + attention; out_proj rides h3 ----
        kkp_cm = tc.tile_pool(name="kk", bufs=1)
        kkp = kkp_cm.__enter__()
        with (
            tc.tile_pool(name="ps_s", bufs=2, space="PSUM") as pss,
            tc.tile_pool(name="ps_o", bufs=2, space="PSUM") as pso,
            tc.tile_pool(name="ps_den", bufs=1, space="PSUM") as psd,
            tc.tile_pool(name="et", bufs=6) as etp,
            tc.tile_pool(name="dn", bufs=3) as dnp,
            tc.tile_pool(name="ln2", bufs=1) as lnp,
            tc.tile_pool(name="ostage", bufs=2) as osp,
        ):

            def emit_qkp(h):
                hsl = slice(h * 128, (h + 1) * 128)
                kk = kkp.tile([128, L], BF16, tag="kk", name=f"kk{h}")
                for ch in range(NCH):
                    csl = slice(ch * 512, (ch + 1) * 512)
                    qps = pmm.tile([128, 512], F32, tag="mm", name=f"qps{h}_{ch}")
                    for kt in range(KT):
                        nc.tensor.matmul(qps[:], wsc["q"][kt][:, hsl], xn[kt][:, csl],
                                         start=(kt == 0), stop=(kt == KT - 1))
                    nc.scalar.activation(qT[h][:, csl], qps[:], AF.Identity,
                                         bias=bqi[h], scale=inv_bc[h])

                    kps = pmm.tile([128, 512], F32, tag="mm", name=f"kps{h}_{ch}")
                    for kt in range(KT):
                        nc.tensor.matmul(kps[:], wsc["k"][kt][:, hsl], xn[kt][:, csl],
                                         start=(kt == 0), stop=(kt == KT - 1))
                    nc.scalar.activation(kk[:, csl], kps[:], AF.Identity,
                                         bias=bko[h], scale=om_bc[h])

                    pps = pmm.tile([128, 512], F32, tag="mm", name=f"pps{h}_{ch}")
                    for kt in range(KT):
                        nc.tensor.matmul(pps[:], wsc["p"][kt][:, hsl], xn[kt][:, csl],
                                         start=(kt == 0), stop=(kt == KT - 1))
                    nc.scalar.activation(geff[h][:, csl], pps[:], AF.Silu, bias=bp[h])
                # smear
                nc.vector.scalar_tensor_tensor(
                    kS[h][:, 1:L], kk[:, 0 : L - 1], ratio_bc[h], kk[:, 1:L],
                    ALU.mult, ALU.add,
                )
                nc.vector.tensor_copy(kS[h][:, 0:1], kk[:, 0:1])

            def emit_attn(h, chs=None):
                hsl = slice(h * 128, (h + 1) * 128)
                for ch in (range(NCH) if chs is None else chs):
                    csl = slice(ch * 512, (ch + 1) * 512)
                    kb_lo = max(0, 4 * ch + 1 - WB[h])
                    kb_hi = 4 * ch + 3
                    ops_ps = pso.tile([128, 512], F32, tag="ops", name=f"ops{h}_{ch}")
                    dsc = psd.tile([128, 512], F32, tag="den", name=f"den{h}_{ch}")
                    den_ps = dsc[:, 0:4]
                    first = {qs: None for qs in range(4)}
                    den_started = False
                    for kb in range(kb_lo, kb_hi + 1):
                        qs0 = max(0, kb - 4 * ch)
                        qs1 = min(4, kb - 4 * ch + WB[h])
                        if qs0 >= qs1:
                            continue
                        nsl = slice(csl.start + qs0 * 128, csl.start + qs1 * 128)
                        esl = slice(qs0 * 128, qs1 * 128)
                        sps = pss.tile([128, 512], F32, tag="sps", name=f"sps{h}_{ch}_{kb}")
                        nc.tensor.matmul(
                            sps[:, esl], kS[h][:, kb * 128 : (kb + 1) * 128],
                            qT[h][:, nsl], start=True, stop=True,
                        )
                        et = etp.tile([128, 512], BF16, tag="et")
                        if h == 0:
                            for qs in range(qs0, qs1):
                                qsl = slice(qs * 128, (qs + 1) * 128)
                                dd = (4 * ch + qs) - kb
                                nc.scalar.activation(
                                    et[:, qsl], sps[:, qsl], AF.Exp,
                                    bias=bias_v[h][dd],
                                )
                        elif h == 1:
                            nc.scalar.activation(
                                et[:, esl], sps[:, esl], AF.Exp,
                                bias=bias_w[4 * ch - kb + 3],
                            )
                        else:
                            nc.scalar.activation(et[:, esl], sps[:, esl], AF.Exp)
                        for qs in range(qs0, qs1):
                            if (4 * ch + qs) == kb:
                                qsl = slice(qs * 128, (qs + 1) * 128)
                                nc.vector.tensor_mul(et[:, qsl], et[:, qsl], tri[:])
                        st = all(first[qs] is None for qs in range(qs0, qs1))
                        for qs in range(qs0, qs1):
                            if first[qs] is None:
                                first[qs] = kb
                        nc.tensor.matmul(
                            ops_ps[:, esl], vtok[kb][:, hsl], et[:, esl],
                            start=st, stop=(kb == kb_hi),
                        )
                        for qs in range(qs0, qs1):
                            qsl = slice(qs * 128, (qs + 1) * 128)
                            nc.tensor.matmul(
                                den_ps[:, qs : qs + 1], et[:, qsl], ones_bf[:],
                                start=(not den_started), stop=(kb == 4 * ch + qs),
                                skip_group_check=True,
                            )
                            den_started = True
                    den_sb = dnp.tile([128, 4], BF16, tag="densb")
                    with nc.allow_low_precision("bf16 den feeds transpose"):
                        nc.vector.tensor_copy(den_sb[:], den_ps[:])
                    dr_ps = dsc[0:1, 64:320].bitcast(BF16)
                    for qs in range(4):
                        nc.tensor.matmul(
                            dr_ps[0:1, qs * 128 : (qs + 1) * 128],
                            den_sb[:, qs : qs + 1], idnb[:],
                            start=(qs == 0), stop=(qs == 3), is_transpose=True,
                            skip_group_check=True,
                        )
                    dinv = dnp.tile([1, 512], BF16, tag="dinv")
                    with nc.allow_low_precision("bf16 1/den feeds a bf16 matmul"):
                        nc.vector.reciprocal(dinv[:], dr_ps[:])
                    dbc_ps = psd.tile([128, 512], F32, tag="dbcps", name=f"dbc{h}_{ch}")
                    nc.tensor.matmul(dbc_ps[:], ones_bfr[:], dinv[:],
                                     start=True, stop=True)
                    dbc = dnp.tile([128, 512], BF16, tag="dbc")
                    with nc.allow_low_precision("bf16 1/den broadcast"):
                        nc.vector.tensor_copy(dbc[:], dbc_ps[:])
                    ozc = dnp.tile([128, 512], BF16, tag="ozc")
                    nc.vector.tensor_mul(ozc[:], ops_ps[:], dbc[:])
                    nc.vector.tensor_mul(geff[h][:, csl], ozc[:], geff[h][:, csl])

                    if h == NH - 1:
                        g = ch
                        for mi in range(4):
                            m = 4 * g + mi
                            msl = slice(m * 128, (m + 1) * 128)
                            for nch2 in range(2):
                                nsl2 = slice(nch2 * 512, (nch2 + 1) * 512)
                                op2 = pmm.tile([128, 512], F32, tag="mm", name=f"op2_{m}_{nch2}")
                                for hh in range(NH):
                                    nc.tensor.matmul(
                                        op2[:], geff[hh][:, msl], woutT[hh][:, nsl2],
                                        start=(hh == 0), stop=(hh == NH - 1),
                                    )
                                osb = osp.tile([128, 512], F32, tag="osb")
                                nc.vector.tensor_copy(osb[:], op2[:])
                                nc.sync.dma_start(
                                    rs_in[g][mi * 128 : (mi + 1) * 128, nsl2], osb[:]
                                )
                        if with_cc:
                            nc.gpsimd.collective_compute(
                                "ReduceScatter", ALU.add,
                                replica_groups=[[0, 1, 2, 3], [4, 5, 6, 7]],
                                ins=[rs_in[g][:, :].opt()],
                                outs=[rs_out[g][:, :].opt()],
                            )
                        else:
                            nc.sync.dma_start(rs_out[g][:, :], rs_in[g][0:128, :])
                        yt = lnp.tile([128, D], F32, tag="yt")
                        nc.sync.dma_start(yt[:], rs_out[g][:, :])
                        bs = lnp.tile([128, 12], F32, tag="bs")
                        nc.vector.bn_stats(bs[:, 0:6], yt[:, 0:512])
                        nc.vector.bn_stats(bs[:, 6:12], yt[:, 512:1024])
                        ag = lnp.tile([128, 2], F32, tag="ag")
                        nc.vector.bn_aggr(ag[:], bs[:])
                        sd2 = lnp.tile([128, 1], F32, tag="sd2")
                        nc.scalar.activation(sd2[:], ag[:, 1:2], AF.Sqrt, bias=eps128[:])
                        rstd2 = lnp.tile([128, 1], F32, tag="rstd2")
                        nc.vector.reciprocal(rstd2[:], sd2[:])
                        nmu2 = lnp.tile([128, 1], F32, tag="nmu2")
                        nc.vector.scalar_tensor_tensor(
                            nmu2[:], ag[:, 0:1], -1.0, rstd2[:], ALU.mult, ALU.mult
                        )
                        t2 = lnp.tile([128, D], F32, tag="t2")
                        nc.scalar.activation(t2[:], yt[:], AF.Identity, bias=nmu2[:], scale=rstd2[:])
                        if ln2_trivial:
                            nc.sync.dma_start(out_d[g * 128 : (g + 1) * 128, :], t2[:])
                        else:
                            t3 = lnp.tile([128, D], F32, tag="t3")
                            nc.vector.tensor_mul(t3[:], t2[:], g2bc[:])
                            nc.vector.tensor_add(t3[:], t3[:], b2bc[:])
                            nc.sync.dma_start(out_d[g * 128 : (g + 1) * 128, :], t3[:])

            # software-pipelined emission: attention(h) interleaves with qkp(h+1)
            emit_qkp(0)
            emit_qkp(1)
            emit_attn(0)
            emit_qkp(2)
            emit_attn(1)
            emit_qkp(3)
            for h in range(NH):
                nc.sync.dma_start(woutT[h][:], wout_d[h * 128 : (h + 1) * 128, :])
            if not ln2_trivial:
                nc.sync.dma_start(g2bc[:], g2bc_d[:, :])
                nc.sync.dma_start(b2bc[:], b2bc_d[:, :])
            for ch in range(NCH):
                emit_attn(2, [ch])
                emit_attn(3, [ch])

        kkp_cm.__exit__(None, None, None)
        wop_cm.__exit__(None, None, None)
        pmm_cm.__exit__(None, None, None)
        wscp_cm.__exit__(None, None, None)
        xnp_cm.__exit__(None, None, None)
        rowp_cm.__exit__(None, None, None)
        qkpA_cm.__exit__(None, None, None)
        dram_cm.__exit__(None, None, None)
        resid_cm.__exit__(None, None, None)
        cp_cm.__exit__(None, None, None)

    _normalize_waits(nc)
    return nc


def _slopes16():
    half = NHEADS // 2
    return np.concatenate(
        [2.0 ** np.linspace(0.0, -8.0, half), np.zeros(NHEADS - half)]
    ).astype(np.float32)


def kernel(x, ln1_g, ln1_b, ln2_g, ln2_b, w_in, w_out, smear_factor, log_scale):
    x = np.asarray(x, np.float32)
    w_in = np.asarray(w_in, np.float32)
    w_out = np.asarray(w_out, np.float32)
    ln1_g = np.asarray(ln1_g, np.float32)
    ln1_b = np.asarray(ln1_b, np.float32)
    ln2_g = np.asarray(ln2_g, np.float32)
    ln2_b = np.asarray(ln2_b, np.float32)
    smear_factor = np.asarray(smear_factor, np.float32)
    log_scale = np.asarray(log_scale, np.float32)

    b1_zero = not np.any(ln1_b)
    ln2_trivial = (not np.any(ln2_b)) and np.all(ln2_g == 1.0)
    key = ("nc", b1_zero, ln2_trivial)
    if key not in _CACHED:
        _CACHED[key] = build(b1_zero=b1_zero, ln2_trivial=ln2_trivial)
    nc = _CACHED[key]

    # fold ln1 gamma into w_in host-side
    wg = w_in * ln1_g[:, None]
    bw = ln1_b @ wg  # [4*DEXP] contribution of ln1 beta

    slopes16 = _slopes16()
    jj = np.arange(128)
    tri = (jj[:, None] <= jj[None, :]).astype(NP_BF16)  # keep j <= i
    idn = np.eye(128, dtype=np.float32)

    in_maps = []
    for c in range(8):
        b, r = divmod(c, 4)
        hs = HGROUPS[r]
        cols = np.concatenate([np.arange(h * 128, (h + 1) * 128) for h in hs])
        sl = slopes16[hs]
        inv = np.exp(-2.0 * log_scale[hs]) / np.sqrt(128.0)
        sg = 1.0 / (1.0 + np.exp(-smear_factor[hs]))
        om = 1.0 - sg
        ratio = np.exp(smear_factor[hs])
        hbc = np.tile(
            np.concatenate([inv, om, ratio]).reshape(1, 3 * NH), (128, 1)
        ).astype(np.float32)
        # per-head ln1-beta bias columns: q scaled by inv, k by om, p raw
        bq = bw[0 * DEXP + cols].reshape(NH, 128)
        bk = bw[1 * DEXP + cols].reshape(NH, 128)
        bpv = bw[3 * DEXP + cols].reshape(NH, 128)
        qkb = np.concatenate(
            [bq.T * inv[None, :], bk.T * om[None, :], bpv.T], axis=1
        ).astype(np.float32)  # [128, 12]
        wv_sl = np.ascontiguousarray(wg[:, 2 * DEXP + cols]).astype(np.float32)
        vcb = np.tile(wv_sl.sum(axis=0, dtype=np.float64).astype(np.float32)[None, :], (128, 1))
        iota_c = np.arange(128, dtype=np.float32)
        bias_cols = [sl[0] * (iota_c - 128 * d - 63) for d in range(NB0)]
        # slot1: one vector per dd = 4*ch - kb in [-3, 15]:
        bias_cols += [sl[1] * (iota_c - 128 * d - 447) for d in range(-3, 16)]
        biasv = np.stack(bias_cols, axis=1).astype(np.float32)
        m = {
            "xt": np.ascontiguousarray(x[b].T).astype(NP_BF16),
            "wq": np.ascontiguousarray(wg[:, 0 * DEXP + cols]).astype(NP_BF16),
            "wk": np.ascontiguousarray(wg[:, 1 * DEXP + cols]).astype(NP_BF16),
            "wv": wv_sl.astype(NP_BF16),
            "wp": np.ascontiguousarray(wg[:, 3 * DEXP + cols]).astype(NP_BF16),
            "wout": np.ascontiguousarray(w_out[cols, :]).astype(NP_BF16),
            "hbc": hbc,
            "qkb": qkb,
            "vcb": vcb.astype(np.float32),
            "biasv": biasv,
            "tri": tri,
            "idn": idn,
            "idnb": idn.astype(NP_BF16),
        }
        if not b1_zero:
            m["bvbc"] = np.tile(bw[2 * DEXP + cols][None, :], (128, 1)).astype(np.float32)
        if not ln2_trivial:
            m["g2bc"] = np.tile(ln2_g.reshape(1, D), (128, 1)).astype(np.float32)
            m["b2bc"] = np.tile(ln2_b.reshape(1, D), (128, 1)).astype(np.float32)
        in_maps.append(m)

    res = None
    last_exc = None
    for _attempt in range(3):
        try:
            res = run_bass_kernel_spmd(nc, in_maps, core_ids=list(range(8)))
            break
        except Exception as e:  # transient axon worker drops; retry
            last_exc = e
            import time as _time

            _time.sleep(2.0)
    if res is None:
        raise last_exc
    _CACHED["last_res"] = res
    out = np.empty((B, L, D), np.float32)
    for c in range(8):
        b, r = divmod(c, 4)
        o = res.results[c]["out"]  # [512, 1024]
        for g in range(NG):
            out[b, 512 * g + 128 * r : 512 * g + 128 * r + 128, :] = o[
                128 * g : 128 * (g + 1), :
            ]
    return out


# revision 60
# speedup vs baseline: 1.3743x; 1.0500x over previous
"""Trainium2 Bass kernel for nn_Block_65755949302136 (dense transformer block).

Sharding: 8 cores = 2 (batch) x 4 (tensor-parallel ranks). Each rank owns 4
heads (2 sloped-ALiBi + 2 zero-slope, balanced), the matching w_in column
slices (q/k/v/p) and w_out row slice. ReduceScatter(add) over each batch
group after out_proj, LN2 computed locally on each rank's 512-row shard.

v2 dataflow (all feature-major, no on-device transposes of activations):
  - LN1 gamma folded into W host-side; beta rides the ACT bias slots.
  - x is centered+normalized ONCE into xn (bf16) via two DVE tensor-tensor
    ops against PE-broadcast rstd rows; q/k/p GEMMs read xn, so no extended
    contraction tile and no per-output rstd multiply.
  - v GEMM runs on RAW x during the centering window (keeps PE busy);
    its -mu*colsum(Wv) correction is a token-major DVE stt against a
    host-sent colsum broadcast, scaled by rstd in the ACT copy.
  - Softmax denominator: ap=1 matmuls (free on PE) accumulate per-query
    den columns in PSUM; one PE transpose + reciprocal + 4 outer-product
    matmuls rebuild the [dh, q] reciprocal broadcast.
  - Slot-0 (steep-slope) heads use a 2-block attention window.
"""

import sys

sys.path.insert(0, "/opt/trn_rl_repo")

import numpy as np

import concourse.bass as bass
import concourse.mybir as mybir
import concourse.tile as tile
from concourse.bass_utils import run_bass_kernel_spmd

F32 = mybir.dt.float32
F32R = mybir.dt.float32r
BF16 = mybir.dt.bfloat16
NP_BF16 = mybir.dt.np(BF16)
AF = mybir.ActivationFunctionType
ALU = mybir.AluOpType

B, L, D, NHEADS, DH = 2, 2048, 1024, 16, 128
DEXP = 2048  # full d_expanded
NH = 4  # heads per core
DL = NH * DH  # 512, local d_expanded slice
KT = D // 128  # 8 k-tiles over d_model
NCH = L // 512  # 4 query chunks
NQT = L // 128  # 16 query tiles
NMT = L // 128  # 16 token tiles
NG = 4  # reduce-scatter groups (512 rows each)

# head assignment: rank r -> [sloped_windowed, sloped_full, zero, zero]
HGROUPS = [[0, 7, 8, 9], [1, 6, 10, 11], [2, 5, 12, 13], [3, 4, 14, 15]]
# per-slot block window (slot0 slopes >= 0.0924: dropped mass < 1e-4 at WB=2)
WB = {0: 2, 1: 16, 2: 16, 3: 16}
NB0 = WB[0]  # slot-0 bias columns in biasv

_CACHED = {}


def _normalize_waits(nc):
    """walrus wait-slot limits are tighter than what Tile emits for some
    instruction classes; move excess sync-waits onto same-engine Drain
    carriers inserted immediately before the instruction."""
    for func in nc.m.functions:
        for blk in func.blocks:
            insts = blk.instructions
            i = 0
            while i < len(insts):
                inst = insts[i]
                si = inst.sync_info
                cap = 1
                if si is not None and len(si.on_wait or []) > cap:
                    waits = list(si.on_wait)
                    excess, keep = waits[:-cap], waits[-cap:]
                    for j, w in enumerate(excess):
                        d = mybir.InstNoOp(
                            name=f"{inst.name}-wsplit{j}",
                            engine=inst.engine,
                            ins=[],
                            outs=[],
                        )
                        d.sync_info = mybir.SyncInfo(on_wait=[w], on_update=[])
                        insts.insert(i, d)
                        nc.register_instruction(d, overwrite=True)
                        i += 1
                    si.on_wait = keep
                i += 1


def build(with_cc=True, b1_zero=True, ln2_trivial=True):
    nc = bass.Bass()

    xt_d = nc.dram_tensor("xt", [D, L], BF16, kind="ExternalInput")
    wq_d = nc.dram_tensor("wq", [D, DL], BF16, kind="ExternalInput")
    wk_d = nc.dram_tensor("wk", [D, DL], BF16, kind="ExternalInput")
    wv_d = nc.dram_tensor("wv", [D, DL], BF16, kind="ExternalInput")
    wp_d = nc.dram_tensor("wp", [D, DL], BF16, kind="ExternalInput")
    wout_d = nc.dram_tensor("wout", [DL, D], BF16, kind="ExternalInput")
    hbc_d = nc.dram_tensor("hbc", [128, 3 * NH], F32, kind="ExternalInput")
    qkb_d = nc.dram_tensor("qkb", [128, 3 * NH], F32, kind="ExternalInput")
    vcb_d = nc.dram_tensor("vcb", [128, DL], F32, kind="ExternalInput")
    biasv_d = nc.dram_tensor("biasv", [128, NB0 + 19], F32, kind="ExternalInput")
    tri_d = nc.dram_tensor("tri", [128, 128], BF16, kind="ExternalInput")
    idn_d = nc.dram_tensor("idn", [128, 128], F32, kind="ExternalInput")
    idnb_d = nc.dram_tensor("idnb", [128, 128], BF16, kind="ExternalInput")
    if not b1_zero:
        bvbc_d = nc.dram_tensor("bvbc", [128, DL], F32, kind="ExternalInput")
    if not ln2_trivial:
        g2bc_d = nc.dram_tensor("g2bc", [128, D], F32, kind="ExternalInput")
        b2bc_d = nc.dram_tensor("b2bc", [128, D], F32, kind="ExternalInput")
    out_d = nc.dram_tensor("out", [NG * 128, D], F32, kind="ExternalOutput")

    with tile.TileContext(nc, pool_alloc_mode="queue") as tc:
        cp_cm = tc.tile_pool(name="const", bufs=1)
        cp = cp_cm.__enter__()

        # ---- tiny constants (DMAs are emitted after the x/wv loads: the
        # sync queue is serial and x gates the whole front of the kernel) ----
        tri = cp.tile([128, 128], BF16, tag="tri")
        idn = cp.tile([128, 128], F32, tag="idn")
        idnb = cp.tile([128, 128], BF16, tag="idnb")
        hbc = cp.tile([128, 3 * NH], F32, tag="hbc")
        qkb = cp.tile([128, 3 * NH], F32, tag="qkb")
        vcb = cp.tile([128, DL], F32, tag="vcb")
        biasv = cp.tile([128, NB0 + 19], F32, tag="biasv")

        def emit_const_dmas():
            nc.sync.dma_start(idnb[:], idnb_d[:, :])
            nc.sync.dma_start(vcb[:], vcb_d[:, :])
            nc.sync.dma_start(hbc[:], hbc_d[:, :])
            nc.sync.dma_start(qkb[:], qkb_d[:, :])
            nc.sync.dma_start(tri[:], tri_d[:, :])
            nc.sync.dma_start(idn[:], idn_d[:, :])
            nc.sync.dma_start(biasv[:], biasv_d[:, :])

        ones_bf = cp.tile([128, 1], BF16, tag="ones_bf")
        nc.gpsimd.memset(ones_bf[:], 1.0)
        ones_bfr = cp.tile([1, 128], BF16, tag="ones_bfr")
        nc.gpsimd.memset(ones_bfr[:], 1.0)
        ones128b = cp.tile([128, 128], BF16, tag="ones128b")
        nc.gpsimd.memset(ones128b[:], 1.0)
        eps128 = cp.tile([128, 1], F32, tag="eps128")
        nc.gpsimd.memset(eps128[:], 1e-5)

        inv_bc = [hbc[:, h : h + 1] for h in range(NH)]
        om_bc = [hbc[:, NH + h : NH + h + 1] for h in range(NH)]
        ratio_bc = [hbc[:, 2 * NH + h : 2 * NH + h + 1] for h in range(NH)]
        bqi = [qkb[:, h : h + 1] for h in range(NH)]
        bko = [qkb[:, NH + h : NH + h + 1] for h in range(NH)]
        bp = [qkb[:, 2 * NH + h : 2 * NH + h + 1] for h in range(NH)]
        bias_v = {0: [biasv[:, d : d + 1] for d in range(NB0)]}
        bias_w = [biasv[:, NB0 + i : NB0 + i + 1] for i in range(19)]

        resid_cm = tc.tile_pool(name="resid", bufs=1)
        resid = resid_cm.__enter__()  # geff + vtok
        dram_cm = tc.tile_pool(name="dram", bufs=1, space="DRAM")
        dram = dram_cm.__enter__()
        qkpA_cm = tc.tile_pool(name="qkpA", bufs=1)
        qkpA = qkpA_cm.__enter__()
        rowp_cm = tc.tile_pool(name="rows", bufs=1)
        rowp = rowp_cm.__enter__()  # rs_bc, mrs_bc, nmu/rs cols, row scratch
        xnp_cm = tc.tile_pool(name="xnp", bufs=1)
        xnp = xnp_cm.__enter__()
        wscp_cm = tc.tile_pool(name="wscp", bufs=1)
        wscp = wscp_cm.__enter__()

        xbp_cm = tc.tile_pool(name="xbp", bufs=1)
        xbp = xbp_cm.__enter__()

        xb = []
        wsc = {}
        wkinds = (("v", wv_d), ("q", wq_d), ("k", wk_d), ("p", wp_d))
        geff = [resid.tile([128, L], BF16, tag=f"geff{h}", name=f"geff{h}") for h in range(NH)]
        vtok = []
        qT = [qkpA.tile([128, L], BF16, tag=f"qT{h}", name=f"qT{h}") for h in range(NH)]
        kS = [qkpA.tile([128, L], BF16, tag=f"kS{h}", name=f"kS{h}") for h in range(NH)]
        xn = [xnp.tile([128, L], BF16, tag=f"xn{kt}", name=f"xn{kt}") for kt in range(KT)]
        rs_in = [dram.tile([512, D], BF16, tag=f"rsin{g}", name=f"rsin{g}") for g in range(NG)]
        rs_out = [dram.tile([128, D], BF16, tag=f"rsout{g}", name=f"rsout{g}") for g in range(NG)]

        rs_bc = rowp.tile([128, L], BF16, tag="rs_bc")
        mrs_bc = rowp.tile([128, L], BF16, tag="mrs_bc")
        # token-major stat columns [128, 16]
        ncol = rowp.tile([128, NMT], F32, tag="ncol")
        sqm = rowp.tile([128, NMT], F32, tag="sqm")
        rs_cols = rowp.tile([128, NMT], F32, tag="rs_cols")
        nmrsc = rowp.tile([128, NMT], F32, tag="nmrsc")
        nmu_cols = ncol

        if not b1_zero:
            bvbc = cp.tile([128, DL], F32, tag="bvbc")
            nc.sync.dma_start(bvbc[:], bvbc_d[:, :])

        # ---- stage A: x/wv DMA + column stats (ap=1 matmuls, ~free on PE)
        # + first v-chain group kt-major to fill PE during the DMA window ----
        NG1 = 6
        vmm_cm = tc.tile_pool(name="ps_vm", bufs=NG1, space="PSUM")
        vmm = vmm_cm.__enter__()
        vps_g1 = [vmm.tile([128, 512], F32, tag="vmm", name=f"vps{m}") for m in range(NG1)]
        with (
            tc.tile_pool(name="ps_sc", bufs=1, space="PSUM") as pscol,
        ):
            stco = pscol.tile([128, 2 * NMT], F32, tag="stco", name="stco")
            scol = stco[:, 0:NMT]
            sqcol = stco[:, NMT : 2 * NMT]
            for kt in range(KT):
                xc = xbp.tile([128, L], BF16, tag=f"xb{kt}", name=f"xb{kt}")
                nc.sync.dma_start(xc[:], xt_d[kt * 128 : (kt + 1) * 128, :])
                xb.append(xc)
                wvt = wscp.tile([128, DL], BF16, tag=f"wv{kt}", name=f"wv{kt}")
                nc.sync.dma_start(wvt[:], wv_d[kt * 128 : (kt + 1) * 128, :])
                wsc.setdefault("v", []).append(wvt)
                xsq = xn[kt]  # xn doubles as the x^2 staging before centering
                nc.scalar.activation(xsq[:], xc[:], AF.Square)
                # NOTE: start=True resets the WHOLE psum bank, so only the
                # very first matmul touching each bank may carry it.
                for m in range(NMT):
                    msl = slice(m * 128, (m + 1) * 128)
                    nc.tensor.matmul(
                        scol[:, m : m + 1], xc[:, msl], ones_bf[:],
                        start=(kt == 0 and m == 0), stop=(kt == KT - 1),
                        skip_group_check=True,
                    )
                    nc.tensor.matmul(
                        sqcol[:, m : m + 1], xsq[:, msl], ones_bf[:],
                        start=False, stop=(kt == KT - 1),
                        skip_group_check=True,
                    )
                for m in range(NG1):
                    nc.tensor.matmul(
                        vps_g1[m][:], xc[:, m * 128 : (m + 1) * 128], wvt[:],
                        start=(kt == 0), stop=(kt == KT - 1),
                    )
            emit_const_dmas()
            for kind, wd in (("q", wq_d), ("k", wk_d), ("p", wp_d)):
                tiles = []
                for kt in range(KT):
                    t = wscp.tile([128, DL], BF16, tag=f"w{kind}{kt}", name=f"w{kind}{kt}")
                    nc.sync.dma_start(t[:], wd[kt * 128 : (kt + 1) * 128, :])
                    tiles.append(t)
                wsc[kind] = tiles

            # column-space LN1 stats: all ops on [128, 16]
            nc.vector.tensor_scalar_mul(ncol[:], scol[:], -1.0 / D)
            nc.vector.tensor_scalar_mul(sqm[:], sqcol[:], 1.0 / D)
            nc.vector.tensor_mul(nmrsc[:], ncol[:], ncol[:])
            nc.vector.tensor_sub(sqm[:], sqm[:], nmrsc[:])  # var
            nc.scalar.activation(sqm[:], sqm[:], AF.Sqrt, bias=eps128[:])
            nc.vector.reciprocal(rs_cols[:], sqm[:])
            nc.vector.tensor_mul(nmrsc[:], ncol[:], rs_cols[:])

        # ---- stage B: broadcasts, remaining v chains, centering ----
        def emit_vpost(m, vps):
            # v += (-mu) * colsum(Wv) (token-major), then *rstd in the copy
            nc.vector.scalar_tensor_tensor(
                vps[:], vcb[:], nmu_cols[:, m : m + 1], vps[:], ALU.mult, ALU.add
            )
            if not b1_zero:
                nc.vector.tensor_add(vps[:], vps[:], bvbc[:])
            vt = resid.tile([128, DL], BF16, tag=f"vtok{m}", name=f"vtok{m}")
            nc.scalar.activation(vt[:], vps[:], AF.Copy, scale=rs_cols[:, m : m + 1])
            vtok.append(vt)

        def emit_vchain(m):
            msl = slice(m * 128, (m + 1) * 128)
            vps = vmm.tile([128, 512], F32, tag="vmm", name=f"vps{m}")
            for kt in range(KT):
                nc.tensor.matmul(vps[:], xb[kt][:, msl], wsc["v"][kt][:],
                                 start=(kt == 0), stop=(kt == KT - 1))
            return vps

        def emit_centering(ch, eng):
            # xn = x * rs_bc, then += mrs_bc in place (bf16 SBUF, 2x on DVE)
            csl = slice(ch * 512, (ch + 1) * 512)
            for kt in range(KT):
                eng.tensor_mul(xn[kt][:, csl], xb[kt][:, csl], rs_bc[:, csl])
                eng.tensor_add(xn[kt][:, csl], xn[kt][:, csl], mrs_bc[:, csl])

        # free three G1 slots, start two G2 chains to cover the broadcast wait
        for m in range(3):
            emit_vpost(m, vps_g1[m])
        vps_pend = [(6, emit_vchain(6)), (7, emit_vchain(7))]

        # rstd / (-mu*rstd) broadcasts: diag(cols) matmul against all-ones;
        # diag construction rides the idle Pool engine
        with (
            tc.tile_pool(name="pbc", bufs=2, space="PSUM") as pbc,
            tc.tile_pool(name="dgp", bufs=4) as dgp,
        ):
            for ch in range(NCH):
                sl = slice(ch * 512, (ch + 1) * 512)
                for si, (src, dst) in enumerate(((rs_cols, rs_bc), (nmrsc, mrs_bc))):
                    bps = pbc.tile([128, 512], F32, tag="bcps", name=f"bc{si}_{ch}")
                    for mi in range(4):
                        m = 4 * ch + mi
                        dg = dgp.tile([128, 128], BF16, tag="dg")
                        nc.vector.tensor_scalar_mul(dg[:], idnb[:], src[:, m : m + 1])
                        nc.tensor.matmul(bps[:, mi * 128 : (mi + 1) * 128],
                                         ones128b[:], dg[:], start=(mi == 0),
                                         stop=(mi == 3), skip_group_check=True)
                    nc.scalar.copy(dst[:, sl], bps[:])

        for m in range(3, NG1):
            emit_vpost(m, vps_g1[m])
        emit_centering(3, nc.gpsimd)  # Pool trails; qkp reaches ch3 last
        for m in range(8, NMT):
            vps_pend.append((m, emit_vchain(m)))
            if len(vps_pend) >= 3:
                emit_vpost(*vps_pend.pop(0))
        while vps_pend:
            emit_vpost(*vps_pend.pop(0))
        for ch in range(3):
            emit_centering(ch, nc.vector)

        vmm_cm.__exit__(None, None, None)
        xbp_cm.__exit__(None, None, None)  # raw x no longer needed
        pmm_cm = tc.tile_pool(name="ps_mm", bufs=2, space="PSUM")
        pmm = pmm_cm.__enter__()

        wop_cm = tc.tile_pool(name="wo", bufs=1)
        wop = wop_cm.__enter__()
        woutT = []
        for h in range(NH):
            t = wop.tile([128, D], BF16, tag=f"woutT{h}", name=f"woutT{h}")
            woutT.append(t)
        if not ln2_trivial:
            g2bc = wop.tile([128, D], F32, tag="g2bc")
            b2bc = wop.tile([128, D], F32, tag="b2bc")

        # ---- stages C-E: head-pipelined qkp + attention; out_proj rides h3 ----
        kkp_cm = tc.tile_pool(name="kk", bufs=1)
        kkp = kkp_cm.__enter__()
        with (
            tc.tile_pool(name="ps_s", bufs=2, space="PSUM") as pss,
            tc.tile_pool(name="ps_o", bufs=2, space="PSUM") as pso,
            tc.tile_pool(name="ps_den", bufs=1, space="PSUM") as psd,
            tc.tile_pool(name="et", bufs=6) as etp,
            tc.tile_pool(name="dn", bufs=3) as dnp,
            tc.tile_pool(name="ln2", bufs=1) as lnp,
            tc.tile_pool(name="ostage", bufs=2) as osp,
        ):

            def gen_qkp(h):
                hsl = slice(h * 128, (h + 1) * 128)
                kk = kkp.tile([128, L], BF16, tag="kk", name=f"kk{h}")
                for ch in range(NCH):
                    csl = slice(ch * 512, (ch + 1) * 512)
                    qps = pmm.tile([128, 512], F32, tag="mm", name=f"qps{h}_{ch}")
                    for kt in range(KT):
                        nc.tensor.matmul(qps[:], wsc["q"][kt][:, hsl], xn[kt][:, csl],
                                         start=(kt == 0), stop=(kt == KT - 1))
                    nc.scalar.activation(qT[h][:, csl], qps[:], AF.Identity,
                                         bias=bqi[h], scale=inv_bc[h])
                    yield

                    kps = pmm.tile([128, 512], F32, tag="mm", name=f"kps{h}_{ch}")
                    for kt in range(KT):
                        nc.tensor.matmul(kps[:], wsc["k"][kt][:, hsl], xn[kt][:, csl],
                                         start=(kt == 0), stop=(kt == KT - 1))
                    nc.scalar.activation(kk[:, csl], kps[:], AF.Identity,
                                         bias=bko[h], scale=om_bc[h])
                    yield

                    pps = pmm.tile([128, 512], F32, tag="mm", name=f"pps{h}_{ch}")
                    for kt in range(KT):
                        nc.tensor.matmul(pps[:], wsc["p"][kt][:, hsl], xn[kt][:, csl],
                                         start=(kt == 0), stop=(kt == KT - 1))
                    nc.scalar.activation(geff[h][:, csl], pps[:], AF.Silu, bias=bp[h])
                    # smear per-chunk so attention can start behind the k ACTs
                    cs, ce = ch * 512, (ch + 1) * 512
                    if ch == 0:
                        nc.vector.tensor_copy(kS[h][:, 0:1], kk[:, 0:1])
                        nc.vector.scalar_tensor_tensor(
                            kS[h][:, 1:512], kk[:, 0:511], ratio_bc[h],
                            kk[:, 1:512], ALU.mult, ALU.add,
                        )
                    else:
                        nc.vector.scalar_tensor_tensor(
                            kS[h][:, cs:ce], kk[:, cs - 1 : ce - 1], ratio_bc[h],
                            kk[:, cs:ce], ALU.mult, ALU.add,
                        )
                    yield

            def gen_attn(h, chs=None):
                hsl = slice(h * 128, (h + 1) * 128)
                for ch in (range(NCH) if chs is None else chs):
                    csl = slice(ch * 512, (ch + 1) * 512)
                    kb_lo = max(0, 4 * ch + 1 - WB[h])
                    kb_hi = 4 * ch + 3
                    ops_ps = pso.tile([128, 512], F32, tag="ops", name=f"ops{h}_{ch}")
                    dsc = psd.tile([128, 512], F32, tag="den", name=f"den{h}_{ch}")
                    den_ps = dsc[:, 0:4]
                    first = {qs: None for qs in range(4)}
                    den_started = False
                    for kb in range(kb_lo, kb_hi + 1):
                        qs0 = max(0, kb - 4 * ch)
                        qs1 = min(4, kb - 4 * ch + WB[h])
                        if qs0 >= qs1:
                            continue
                        nsl = slice(csl.start + qs0 * 128, csl.start + qs1 * 128)
                        esl = slice(qs0 * 128, qs1 * 128)
                        sps = pss.tile([128, 512], F32, tag="sps", name=f"sps{h}_{ch}_{kb}")
                        nc.tensor.matmul(
                            sps[:, esl], kS[h][:, kb * 128 : (kb + 1) * 128],
                            qT[h][:, nsl], start=True, stop=True,
                        )
                        et = etp.tile([128, 512], BF16, tag="et")
                        if h == 0:
                            for qs in range(qs0, qs1):
                                qsl = slice(qs * 128, (qs + 1) * 128)
                                dd = (4 * ch + qs) - kb
                                nc.scalar.activation(
                                    et[:, qsl], sps[:, qsl], AF.Exp,
                                    bias=bias_v[h][dd],
                                )
                        elif h == 1:
                            nc.scalar.activation(
                                et[:, esl], sps[:, esl], AF.Exp,
                                bias=bias_w[4 * ch - kb + 3],
                            )
                        else:
                            nc.scalar.activation(et[:, esl], sps[:, esl], AF.Exp)
                        for qs in range(qs0, qs1):
                            if (4 * ch + qs) == kb:
                                qsl = slice(qs * 128, (qs + 1) * 128)
                                nc.vector.tensor_mul(et[:, qsl], et[:, qsl], tri[:])
                        yield
                        st = all(first[qs] is None for qs in range(qs0, qs1))
                        for qs in range(qs0, qs1):
                            if first[qs] is None:
                                first[qs] = kb
                        nc.tensor.matmul(
                            ops_ps[:, esl], vtok[kb][:, hsl], et[:, esl],
                            start=st, stop=(kb == kb_hi),
                        )
                        for qs in range(qs0, qs1):
                            qsl = slice(qs * 128, (qs + 1) * 128)
                            nc.tensor.matmul(
                                den_ps[:, qs : qs + 1], et[:, qsl], ones_bf[:],
                                start=(not den_started), stop=(kb == 4 * ch + qs),
                                skip_group_check=True,
                            )
                            den_started = True
                        yield
                    den_sb = dnp.tile([128, 4], BF16, tag="densb")
                    with nc.allow_low_precision("bf16 den feeds transpose"):
                        nc.vector.tensor_copy(den_sb[:], den_ps[:])
                    dr_ps = dsc[0:1, 64:320].bitcast(BF16)
                    for qs in range(4):
                        nc.tensor.matmul(
                            dr_ps[0:1, qs * 128 : (qs + 1) * 128],
                            den_sb[:, qs : qs + 1], idnb[:],
                            start=(qs == 0), stop=(qs == 3), is_transpose=True,
                            skip_group_check=True,
                        )
                    dinv = dnp.tile([1, 512], BF16, tag="dinv")
                    with nc.allow_low_precision("bf16 1/den feeds a bf16 matmul"):
                        nc.vector.reciprocal(dinv[:], dr_ps[:])
                    dbc_ps = psd.tile([128, 512], F32, tag="dbcps", name=f"dbc{h}_{ch}")
                    nc.tensor.matmul(dbc_ps[:], ones_bfr[:], dinv[:],
                                     start=True, stop=True)
                    dbc = dnp.tile([128, 512], BF16, tag="dbc")
                    with nc.allow_low_precision("bf16 1/den broadcast"):
                        nc.vector.tensor_copy(dbc[:], dbc_ps[:])
                    ozc = dnp.tile([128, 512], BF16, tag="ozc")
                    nc.vector.tensor_mul(ozc[:], ops_ps[:], dbc[:])
                    nc.vector.tensor_mul(geff[h][:, csl], ozc[:], geff[h][:, csl])
                    yield

            def emit_attn(h, chs=None):
                for _ in gen_attn(h, chs):
                    pass

            def interleave(main, bg, k):
                # advance bg ~k steps per main step; exhaust both
                carry = 0.0
                done = False
                for _ in main:
                    carry += k
                    while carry >= 1.0 and not done:
                        carry -= 1.0
                        try:
                            next(bg)
                        except StopIteration:
                            done = True
                while not done:
                    try:
                        next(bg)
                    except StopIteration:
                        done = True

            def outproj_chain(g, mi, nch2, eng_act):
                def emit():
                    m = 4 * g + mi
                    msl = slice(m * 128, (m + 1) * 128)
                    nsl2 = slice(nch2 * 512, (nch2 + 1) * 512)
                    op2 = pmm.tile([128, 512], F32, tag="mm", name=f"op2_{m}_{nch2}")
                    for hh in range(NH):
                        nc.tensor.matmul(
                            op2[:], geff[hh][:, msl], woutT[hh][:, nsl2],
                            start=(hh == 0), stop=(hh == NH - 1),
                        )
                    osb = osp.tile([128, 512], BF16, tag="osb")
                    with nc.allow_low_precision("bf16 residual staging"):
                        if eng_act:
                            nc.scalar.copy(osb[:], op2[:])
                        else:
                            nc.vector.tensor_copy(osb[:], op2[:])
                    nc.sync.dma_start(
                        rs_in[g][mi * 128 : (mi + 1) * 128, nsl2], osb[:]
                    )

                return emit

            def outproj_closures(g):
                return [
                    outproj_chain(g, mi, nch2, eng_act=False)
                    for mi in range(4)
                    for nch2 in range(2)
                ]

            def emit_outproj_fin(g):
                    if True:
                        if with_cc:
                            nc.gpsimd.collective_compute(
                                "ReduceScatter", ALU.add,
                                replica_groups=[[0, 1, 2, 3], [4, 5, 6, 7]],
                                ins=[rs_in[g][:, :].opt()],
                                outs=[rs_out[g][:, :].opt()],
                            )
                        else:
                            nc.sync.dma_start(rs_out[g][:, :], rs_in[g][0:128, :])
                        yt = lnp.tile([128, D], BF16, tag="yt")
                        nc.sync.dma_start(yt[:], rs_out[g][:, :])
                        bs = lnp.tile([128, 12], F32, tag="bs")
                        nc.vector.bn_stats(bs[:, 0:6], yt[:, 0:512])
                        nc.vector.bn_stats(bs[:, 6:12], yt[:, 512:1024])
                        ag = lnp.tile([128, 2], F32, tag="ag")
                        nc.vector.bn_aggr(ag[:], bs[:])
                        sd2 = lnp.tile([128, 1], F32, tag="sd2")
                        nc.scalar.activation(sd2[:], ag[:, 1:2], AF.Sqrt, bias=eps128[:])
                        rstd2 = lnp.tile([128, 1], F32, tag="rstd2")
                        nc.vector.reciprocal(rstd2[:], sd2[:])
                        nmu2 = lnp.tile([128, 1], F32, tag="nmu2")
                        nc.vector.scalar_tensor_tensor(
                            nmu2[:], ag[:, 0:1], -1.0, rstd2[:], ALU.mult, ALU.mult
                        )
                        t2 = lnp.tile([128, D], F32, tag="t2")
                        nc.scalar.activation(t2[:], yt[:], AF.Identity, bias=nmu2[:], scale=rstd2[:])
                        if ln2_trivial:
                            nc.sync.dma_start(out_d[g * 128 : (g + 1) * 128, :], t2[:])
                        else:
                            t3 = lnp.tile([128, D], F32, tag="t3")
                            nc.vector.tensor_mul(t3[:], t2[:], g2bc[:])
                            nc.vector.tensor_add(t3[:], t3[:], b2bc[:])
                            nc.sync.dma_start(out_d[g * 128 : (g + 1) * 128, :], t3[:])

            # software-pipelined emission: attention(h) kb-steps ride inside
            # the qkp(h+1) GEMM phase so exps hide under dense matmul cover
            for _ in gen_qkp(0):
                pass
            interleave(gen_qkp(1), gen_attn(0), k=6.4)
            interleave(gen_qkp(2), gen_attn(1), k=7.5)
            for h in range(NH):
                nc.sync.dma_start(woutT[h][:], wout_d[h * 128 : (h + 1) * 128, :])
            if not ln2_trivial:
                nc.sync.dma_start(g2bc[:], g2bc_d[:, :])
                nc.sync.dma_start(b2bc[:], b2bc_d[:, :])
            for _ in gen_qkp(3):
                pass
            # stagger: attn2/attn3 of chunk ch+1 cover the latency of chunk
            # ch's geff[3] chain before its out_proj consumes it
            emit_attn(2, [0])
            emit_attn(3, [0])
            for ch in range(1, NCH):
                emit_attn(2, [ch])
                emit_attn(3, [ch])
                for c in outproj_closures(ch - 1):
                    c()
                emit_outproj_fin(ch - 1)
            for c in outproj_closures(NCH - 1):
                c()
            emit_outproj_fin(NCH - 1)

        kkp_cm.__exit__(None, None, None)
        wop_cm.__exit__(None, None, None)
        pmm_cm.__exit__(None, None, None)
        wscp_cm.__exit__(None, None, None)
        xnp_cm.__exit__(None, None, None)
        rowp_cm.__exit__(None, None, None)
        qkpA_cm.__exit__(None, None, None)
        dram_cm.__exit__(None, None, None)
        resid_cm.__exit__(None, None, None)
        cp_cm.__exit__(None, None, None)

    _normalize_waits(nc)
    return nc


def _slopes16():
    half = NHEADS // 2
    return np.concatenate(
        [2.0 ** np.linspace(0.0, -8.0, half), np.zeros(NHEADS - half)]
    ).astype(np.float32)


def kernel(x, ln1_g, ln1_b, ln2_g, ln2_b, w_in, w_out, smear_factor, log_scale):
    x = np.asarray(x, np.float32)
    w_in = np.asarray(w_in, np.float32)
    w_out = np.asarray(w_out, np.float32)
    ln1_g = np.asarray(ln1_g, np.float32)
    ln1_b = np.asarray(ln1_b, np.float32)
    ln2_g = np.asarray(ln2_g, np.float32)
    ln2_b = np.asarray(ln2_b, np.float32)
    smear_factor = np.asarray(smear_factor, np.float32)
    log_scale = np.asarray(log_scale, np.float32)

    b1_zero = not np.any(ln1_b)
    ln2_trivial = (not np.any(ln2_b)) and np.all(ln2_g == 1.0)
    key = ("nc", b1_zero, ln2_trivial)
    if key not in _CACHED:
        _CACHED[key] = build(b1_zero=b1_zero, ln2_trivial=ln2_trivial)
    nc = _CACHED[key]

    # fold ln1 gamma into w_in host-side
    wg = w_in * ln1_g[:, None]
    bw = ln1_b @ wg  # [4*DEXP] contribution of ln1 beta

    slopes16 = _slopes16()
    jj = np.arange(128)
    tri = (jj[:, None] <= jj[None, :]).astype(NP_BF16)  # keep j <= i
    idn = np.eye(128, dtype=np.float32)

    in_maps = []
    for c in range(8):
        b, r = divmod(c, 4)
        hs = HGROUPS[r]
        cols = np.concatenate([np.arange(h * 128, (h + 1) * 128) for h in hs])
        sl = slopes16[hs]
        inv = np.exp(-2.0 * log_scale[hs]) / np.sqrt(128.0)
        sg = 1.0 / (1.0 + np.exp(-smear_factor[hs]))
        om = 1.0 - sg
        ratio = np.exp(smear_factor[hs])
        hbc = np.tile(
            np.concatenate([inv, om, ratio]).reshape(1, 3 * NH), (128, 1)
        ).astype(np.float32)
        # per-head ln1-beta bias columns: q scaled by inv, k by om, p raw
        bq = bw[0 * DEXP + cols].reshape(NH, 128)
        bk = bw[1 * DEXP + cols].reshape(NH, 128)
        bpv = bw[3 * DEXP + cols].reshape(NH, 128)
        qkb = np.concatenate(
            [bq.T * inv[None, :], bk.T * om[None, :], bpv.T], axis=1
        ).astype(np.float32)  # [128, 12]
        wv_sl = np.ascontiguousarray(wg[:, 2 * DEXP + cols]).astype(np.float32)
        vcb = np.tile(wv_sl.sum(axis=0, dtype=np.float64).astype(np.float32)[None, :], (128, 1))
        iota_c = np.arange(128, dtype=np.float32)
        bias_cols = [sl[0] * (iota_c - 128 * d - 63) for d in range(NB0)]
        # slot1: one vector per dd = 4*ch - kb in [-3, 15]:
        bias_cols += [sl[1] * (iota_c - 128 * d - 447) for d in range(-3, 16)]
        biasv = np.stack(bias_cols, axis=1).astype(np.float32)
        m = {
            "xt": np.ascontiguousarray(x[b].T).astype(NP_BF16),
            "wq": np.ascontiguousarray(wg[:, 0 * DEXP + cols]).astype(NP_BF16),
            "wk": np.ascontiguousarray(wg[:, 1 * DEXP + cols]).astype(NP_BF16),
            "wv": wv_sl.astype(NP_BF16),
            "wp": np.ascontiguousarray(wg[:, 3 * DEXP + cols]).astype(NP_BF16),
            "wout": np.ascontiguousarray(w_out[cols, :]).astype(NP_BF16),
            "hbc": hbc,
            "qkb": qkb,
            "vcb": vcb.astype(np.float32),
            "biasv": biasv,
            "tri": tri,
            "idn": idn,
            "idnb": idn.astype(NP_BF16),
        }
        if not b1_zero:
            m["bvbc"] = np.tile(bw[2 * DEXP + cols][None, :], (128, 1)).astype(np.float32)
        if not ln2_trivial:
            m["g2bc"] = np.tile(ln2_g.reshape(1, D), (128, 1)).astype(np.float32)
            m["b2bc"] = np.tile(ln2_b.reshape(1, D), (128, 1)).astype(np.float32)
        in_maps.append(m)

    res = None
    last_exc = None
    for _attempt in range(3):
        try:
            res = run_bass_kernel_spmd(nc, in_maps, core_ids=list(range(8)))
            break
        except Exception as e:  # transient axon worker drops; retry
            last_exc = e
            import time as _time

            _time.sleep(2.0)
    if res is None:
        raise last_exc
    _CACHED["last_res"] = res
    out = np.empty((B, L, D), np.float32)
    for c in range(8):
        b, r = divmod(c, 4)
        o = res.results[c]["out"]  # [512, 1024]
        for g in range(NG):
            out[b, 512 * g + 128 * r : 512 * g + 128 * r + 128, :] = o[
                128 * g : 128 * (g + 1), :
            ]
    return out
